# revision 5
# baseline (speedup 1.0000x reference)
"""Trainium2 Bass kernel: 3-layer GAT message passing, 8-core SPMD.

Dst-sharded edge phase (dma_gather + one-hot scatter matmuls into PSUM;
self-loop rows bulk-loaded, bypassing gather descriptors), node-sharded
table build interleaved into the previous edge phase, split AllGathers.
"""
"""GAT message-passing kernel for TRN2, 8-core SPMD.

Per layer:
  node phase (node-sharded): table rows [h' | alpha_s | alpha_d | pad] for own
  nodes via PE matmuls with host-packed rhs [W | U_s | U_d].
  AllGather table -> full gather table in every core's DRAM.
  edge phase (dst-sharded): per dst-window of 128 nodes, dma_gather rows for
  the window's edges (src-indexed, int16 lo/hi split), build one-hot
  scatter/expand matrices on DVE, expand alpha_d via PE matmul,
  w = max(exp(e), exp(slope*e)) == exp(leaky_relu(e)), weighted messages
  scatter-accumulated into PSUM via one-hot matmul, normalized by the
  softmax denominator (segment max subtraction dropped; shift-invariant).
"""

import math
from dataclasses import dataclass, field

import numpy as np
import ml_dtypes

import concourse.bass as bass
import concourse.bacc as bacc
import concourse.mybir as mybir
import concourse.tile as tile
from concourse import library_config

F32 = mybir.dt.float32
BF16 = mybir.dt.bfloat16
I16 = mybir.dt.int16
AF = mybir.ActivationFunctionType
ALU = mybir.AluOpType
NEG_SLOPE = 0.2

bf = ml_dtypes.bfloat16


@dataclass
class Cfg:
    N: int = 50000
    n_cores: int = 8
    IN: int = 256
    HID: int = 128
    OUT: int = 128
    TSPLIT: int = 32768
    K_FUSE: int = 8
    n_queues: int = 4

    @property
    def n_loc(self):
        return self.N // self.n_cores

    @property
    def n_win(self):
        return (self.n_loc + 127) // 128

    def win_size(self, t):
        return min(128, self.n_loc - t * 128)

    @property
    def WA(self):
        return (self.n_win + 1) // 2

    @property
    def nA(self):
        return min(self.WA * 128, self.n_loc)

    @property
    def nB(self):
        return self.n_loc - self.nA

    @property
    def layers(self):
        hid, out = self.HID, self.OUT
        ls = []
        for (H, C, in_ch) in ((4, hid // 2, hid), (2, out, 2 * hid), (1, out, out)):
            HC = H * C
            row = HC + 2 * H
            row_pad = ((row * 2 + 255) // 256) * 256 // 2
            ls.append(dict(H=H, C=C, in_ch=in_ch, HC=HC, row=row_pad,
                           as_off=HC, ad_off=HC + H))
        return ls


# ---------------------------------------------------------------- host plan

@dataclass
class Plan:
    T_lo: list
    T_hi: list
    idx_lo: list
    idx_hi: list
    dstloc_col: list
    dstloc_row: list
    NT: int = 0
    cols_lo: int = 0
    cols_hi: int = 0
    win_tile_off: list = field(default_factory=list)
    win_lo_off: list = field(default_factory=list)
    win_hi_off: list = field(default_factory=list)


def plan_edges(edge_index, cfg: Cfg) -> Plan:
    N, C = cfg.N, cfg.n_cores
    n_loc = cfg.n_loc
    src = np.asarray(edge_index[0], np.int64)      # self loops handled by a
    dst = np.asarray(edge_index[1], np.int64)      # direct per-window load
    core = dst // n_loc
    win = (dst % n_loc) // 128
    dloc = (dst % n_loc) % 128
    s_core = src // n_loc
    s_loc = src % n_loc
    lo = s_loc < cfg.nA
    idx_a = s_core * cfg.nA + s_loc
    idx_b = s_core * cfg.nB + (s_loc - cfg.nA)

    W = cfg.n_win
    buckets = {}
    for c in range(C):
        m_c = core == c
        for t in range(W):
            m = m_c & (win == t)
            ml = m & lo
            mh = m & ~lo
            sl, dll = idx_a[ml], dloc[ml]
            o = np.argsort(sl, kind="stable")
            buckets[(c, t, 0)] = (sl[o], dll[o])
            sh, dlh = idx_b[mh], dloc[mh]
            o = np.argsort(sh, kind="stable")
            buckets[(c, t, 1)] = (sh[o], dlh[o])

    T_lo = [max(math.ceil(len(buckets[(c, t, 0)][0]) / 128) for c in range(C))
            for t in range(W)]
    T_hi = [max(math.ceil(len(buckets[(c, t, 1)][0]) / 128) for c in range(C))
            for t in range(W)]

    NT = sum(T_lo) + sum(T_hi) + W  # +1 self tile per window
    p = Plan(T_lo=T_lo, T_hi=T_hi, idx_lo=[], idx_hi=[], dstloc_col=[],
             dstloc_row=[], NT=NT, cols_lo=max(sum(T_lo) * 8, 8),
             cols_hi=max(sum(T_hi) * 8, 8))
    off = olo = ohi = 0
    for t in range(W):
        p.win_tile_off.append(off)
        p.win_lo_off.append(olo)
        p.win_hi_off.append(ohi)
        off += T_lo[t] + T_hi[t] + 1
        olo += T_lo[t] * 8
        ohi += T_hi[t] * 8

    for c in range(C):
        ilo = np.zeros((128, p.cols_lo), np.int16)
        ihi = np.zeros((128, p.cols_hi), np.int16)
        dcol = np.full((128, NT), -1.0, np.float32)
        for t in range(W):
            for half, (idx_arr, col_off, Tn) in enumerate(
                ((ilo, p.win_lo_off[t], T_lo[t]),
                 (ihi, p.win_hi_off[t], T_hi[t]))):
                if Tn == 0:
                    continue
                s, dl = buckets[(c, t, half)]
                n = Tn * 128
                si = np.zeros(n, np.int64)
                si[:len(s)] = s
                dli = np.full(n, -1.0, np.float32)
                dli[:len(dl)] = dl
                blk = si.astype(np.int16).reshape(Tn * 8, 16).T
                idx_arr[:, col_off:col_off + Tn * 8] = np.tile(blk, (8, 1))
                tb = p.win_tile_off[t] + (0 if half == 0 else T_lo[t])
                dcol[:, tb:tb + Tn] = dli.reshape(Tn, 128).T
            ts = p.win_tile_off[t] + T_lo[t] + T_hi[t]
            nn_w = min(128, n_loc - t * 128)
            selfcol = np.full(128, -1.0, np.float32)
            selfcol[:nn_w] = np.arange(nn_w)
            dcol[:, ts] = selfcol
        p.idx_lo.append(ilo)
        p.idx_hi.append(ihi)
        p.dstloc_col.append(dcol.astype(bf))
        p.dstloc_row.append(dcol.T.reshape(1, NT * 128).astype(bf))
    return p


# ------------------------------------------------------------- host weights

def prep_weights(inp, cfg: Cfg):
    out = {}
    for li, (wk, ak, dk) in enumerate(
            (("g1_W", "g1_as", "g1_ad"), ("g2_W", "g2_as", "g2_ad"),
             ("g3_W", "g3_as", "g3_ad"))):
        L = cfg.layers[li]
        Wm = np.asarray(inp[wk], np.float32)
        a_s = np.asarray(inp[ak], np.float32)
        a_d = np.asarray(inp[dk], np.float32)
        H, Cc = L["H"], L["C"]
        U_s = np.stack([Wm[:, h * Cc:(h + 1) * Cc] @ a_s[h] for h in range(H)], 1)
        U_d = np.stack([Wm[:, h * Cc:(h + 1) * Cc] @ a_d[h] for h in range(H)], 1)
        out[f"WG{li+1}"] = np.concatenate([Wm, U_s, U_d], 1).astype(bf)
        out[f"bG{li+1}"] = np.ascontiguousarray(np.broadcast_to(
            np.asarray(inp[f"g{li+1}_b"], np.float32)[None, :],
            (128, len(inp[f"g{li+1}_b"]))))
    out["Wm1"] = np.asarray(inp["W1"], np.float32).astype(bf)
    out["Wm2"] = np.asarray(inp["W2"], np.float32).astype(bf)
    out["b1c"] = np.ascontiguousarray(np.asarray(inp["b1"], np.float32)[:, None])
    out["b2c"] = np.ascontiguousarray(np.asarray(inp["b2"], np.float32)[:, None])
    out["iota_row"] = np.ascontiguousarray(
        np.broadcast_to(np.arange(128, dtype=np.float32), (128, 128))).astype(bf)
    out["iota_col"] = np.arange(128, dtype=np.float32)[:, None].astype(bf)
    out["ident"] = np.eye(128, dtype=np.float32).astype(bf)
    return out


# ---------------------------------------------------------------- builder

def build(nc, cfg: Cfg, p: Plan, dbg=False):
    W = cfg.n_win
    n_loc = cfg.n_loc
    Ls = cfg.layers
    MAXT = max(p.T_lo[t] + p.T_hi[t] for t in range(W)) + 1
    MAXHC = max(L["HC"] for L in Ls)
    MAXH = max(L["H"] for L in Ls)

    def din(name, shape, dt):
        return nc.dram_tensor(name, list(shape), dt, kind="ExternalInput")

    xT = din("xT", (cfg.IN, n_loc), BF16)
    Wm1 = din("Wm1", (cfg.IN, cfg.HID), BF16)
    Wm2 = din("Wm2", (cfg.HID, cfg.HID), BF16)
    b1c = din("b1c", (cfg.HID, 1), F32)
    b2c = din("b2c", (cfg.HID, 1), F32)
    WG = [din(f"WG{i+1}", (Ls[i]["in_ch"], Ls[i]["HC"] + 2 * Ls[i]["H"]), BF16)
          for i in range(3)]
    bG = [din(f"bG{i+1}", (128, Ls[i]["HC"] if i == 0 else Ls[i]["C"]), F32)
          for i in range(3)]
    iota_row_d = din("iota_row", (128, 128), BF16)
    iota_col_d = din("iota_col", (128, 1), BF16)
    ident_d = din("ident", (128, 128), BF16)
    idx_lo_d = din("idx_lo", (128, p.cols_lo), I16)
    idx_hi_d = din("idx_hi", (128, p.cols_hi), I16)
    dstcol_d = din("dstcol", (128, p.NT), BF16)
    out_d = nc.dram_tensor("out", [n_loc, cfg.OUT], F32, kind="ExternalOutput")
    dbg_d = {}
    if dbg:
        L0 = Ls[0]
        T0 = p.T_lo[0] + p.T_hi[0]
        for nm, shape in (("d_piece0", (cfg.HID, n_loc)),
                          ("d_tin0", (n_loc, L0["row"])),
                          ("d_tag0", (cfg.N, L0["row"])),
                          ("d_g0", (128, T0 * L0["row"])),
                          ("d_oT0", (128, T0 * 128)),
                          ("d_oD0", (128, T0 * 128)),
                          ("d_esb0", (128, T0 * L0["H"])),
                          ("d_msg0", (128, T0 * (L0["HC"] + L0["H"]))),
                          ("d_psw0", (128, L0["HC"] + L0["H"])),
                          ("d_ad0", (128, L0["H"]))):
            dt = BF16 if nm in ("d_tin0", "d_tag0", "d_g0", "d_oT0", "d_oD0",
                                "d_msg0", "d_piece0", "d_ad0") else F32
            dbg_d[nm] = nc.dram_tensor(nm, list(shape), dt, kind="ExternalOutput")

    rep = [list(range(cfg.n_cores))]

    with tile.TileContext(nc) as tc:
        with (
            tc.tile_pool(name="const", bufs=1) as cpool,
            tc.tile_pool(name="gat", bufs=3) as gpool,
            tc.tile_pool(name="oh", bufs=3) as opool,
            tc.tile_pool(name="msg", bufs=2) as mpool,
            tc.tile_pool(name="small", bufs=4) as spool,
            tc.tile_pool(name="nodes", bufs=3) as npool,
            tc.tile_pool(name="psA", bufs=getattr(cfg, "psA_bufs", 2), space="PSUM") as psA,
            tc.tile_pool(name="psB", bufs=getattr(cfg, "psB_bufs", 2), space="PSUM") as psB,
            tc.tile_pool(name="psC", bufs=2, space="PSUM") as psC,
            tc.tile_pool(name="dram", bufs=1, space="DRAM") as dpool,
        ):
            nc.gpsimd.load_library(library_config.mlp)
            gq = [0]  # round-robin SWDGE queue counter for gathers

            def load_const(handle, shape, dtp, tag):
                t = cpool.tile(list(shape), dtp, tag=tag, name=tag)
                nc.sync.dma_start(t[:], handle[:])
                return t

            iota_row = load_const(iota_row_d, (128, 128), BF16, "iota_row")
            iota_col = load_const(iota_col_d, (128, 1), BF16, "iota_col")
            ident = load_const(ident_d, (128, 128), BF16, "ident")
            idx_lo = load_const(idx_lo_d, (128, p.cols_lo), I16, "idx_lo")
            idx_hi = load_const(idx_hi_d, (128, p.cols_hi), I16, "idx_hi")
            dstcol = load_const(dstcol_d, (128, p.NT), BF16, "dstcol")
            wg_sb = []
            for i in range(3):
                L = Ls[i]
                kch = L["in_ch"] // 128
                t = cpool.tile([128, kch, L["HC"] + 2 * L["H"]], BF16, tag=f"wg{i}",
                               name=f"wg{i}")
                for k in range(kch):
                    nc.sync.dma_start(t[:, k, :], WG[i][k * 128:(k + 1) * 128, :])
                wg_sb.append(t)
            bg_sb = [load_const(bG[i], (128, Ls[i]["HC"] if i == 0 else Ls[i]["C"]),
                                F32, f"bg{i}") for i in range(3)]
            wm1 = cpool.tile([128, 2, cfg.HID], BF16, tag="wm1")
            for k in range(2):
                nc.sync.dma_start(wm1[:, k, :], Wm1[k * 128:(k + 1) * 128, :])
            wm2 = load_const(Wm2, (cfg.HID, cfg.HID), BF16, "wm2")
            b1s = load_const(b1c, (cfg.HID, 1), F32, "b1s")
            b2s = load_const(b2c, (cfg.HID, 1), F32, "b2s")

            piece = [dpool.tile([cfg.HID, n_loc], BF16, tag="piece0",
                                name="piece0"),
                     dpool.tile([Ls[0]["HC"], n_loc], BF16, tag="piece1",
                                name="piece1"),
                     dpool.tile([Ls[1]["C"], n_loc], BF16, tag="piece2",
                                name="piece2")]
            tbl_inA = [dpool.tile([cfg.nA, Ls[i]["row"]], BF16, tag=f"tinA{i}",
                                  name=f"tinA{i}")
                       for i in range(3)]
            tbl_inB = [dpool.tile([cfg.nB, Ls[i]["row"]], BF16, tag=f"tinB{i}",
                                  name=f"tinB{i}")
                       for i in range(3)]
            tbl_agA = [dpool.tile([cfg.nA * cfg.n_cores, Ls[i]["row"]], BF16,
                                  tag=f"tagA{i}", name=f"tagA{i}")
                       for i in range(3)]
            tbl_agB = [dpool.tile([cfg.nB * cfg.n_cores, Ls[i]["row"]], BF16,
                                  tag=f"tagB{i}", name=f"tagB{i}")
                       for i in range(3)]

            # ================= MLP (node-sharded) =================
            CH = 512
            nch = math.ceil(n_loc / CH)
            for j in range(nch):
                n0 = j * CH
                nn = min(CH, n_loc - n0)
                xt = npool.tile([128, 2, CH], BF16, tag="xt")
                for k in range(2):
                    nc.sync.dma_start(xt[:, k, :nn],
                                      xT[k * 128:(k + 1) * 128, n0:n0 + nn])
                ps = psC.tile([128, CH], F32, tag="mm")
                for k in range(2):
                    nc.tensor.matmul(ps[:, :nn], wm1[:, k, :], xt[:, k, :nn],
                                     start=(k == 0), stop=(k == 1))
                h1 = npool.tile([128, CH], BF16, tag="h1")
                nc.scalar.activation(h1[:, :nn], ps[:, :nn], AF.Relu,
                                     bias=b1s[:, 0:1])
                ps2 = psC.tile([128, CH], F32, tag="mm")
                nc.tensor.matmul(ps2[:, :nn], wm2[:, :], h1[:, :nn],
                                 start=True, stop=True)
                h2 = npool.tile([128, CH], BF16, tag="h2")
                nc.scalar.activation(h2[:, :nn], ps2[:, :nn], AF.Relu,
                                     bias=b2s[:, 0:1])
                nc.sync.dma_start(piece[0][:, n0:n0 + nn], h2[:, :nn])

            ad_tiles = {}

            def node_chunk(li, j):
                L = Ls[li]
                kch = L["in_ch"] // 128
                NCOL = L["HC"] + 2 * L["H"]
                ROW = L["row"]
                n0 = j * 128
                nn = cfg.win_size(j)
                lh = npool.tile([128, kch, 128], BF16, tag="lh", name="lh")
                for k in range(kch):
                    nc.sync.dma_start(
                        lh[:, k, :nn],
                        piece[li][k * 128:(k + 1) * 128, n0:n0 + nn])
                ps = psC.tile([128, NCOL], F32, tag="mm", name="psn")
                for k in range(kch):
                    nc.tensor.matmul(ps[:nn, :], lh[:, k, :nn],
                                     wg_sb[li][:, k, :],
                                     start=(k == 0), stop=(k == kch - 1))
                tb = npool.tile([128, ROW], BF16, tag="tb", name="tb")
                nc.vector.tensor_copy(tb[:nn, :NCOL], ps[:nn, :])
                if j < cfg.WA:
                    nc.sync.dma_start(
                        tbl_inA[li][n0:n0 + nn, :NCOL], tb[:nn, :NCOL])
                else:
                    m0 = n0 - cfg.nA
                    nc.sync.dma_start(
                        tbl_inB[li][m0:m0 + nn, :NCOL], tb[:nn, :NCOL])

            def emit_ag(li, half):
                if half == 0:
                    nc.gpsimd.collective_compute(
                        "AllGather", ALU.bypass, replica_groups=rep,
                        ins=[tbl_inA[li].opt()], outs=[tbl_agA[li].opt()])
                else:
                    nc.gpsimd.collective_compute(
                        "AllGather", ALU.bypass, replica_groups=rep,
                        ins=[tbl_inB[li].opt()], outs=[tbl_agB[li].opt()])

            def emit_ad(li):
                L = Ls[li]
                H = L["H"]
                ad_all = spool.tile([128, W, MAXH], BF16, tag="ad_all",
                                    name="ad_all", bufs=2)
                ad_tiles[li] = ad_all
                nc.vector.memset(ad_all[:], 0.0)
                ad_fA = tbl_inA[li][:, L["ad_off"]:L["ad_off"] + H]
                nc.sync.dma_start(
                    ad_all[:, :cfg.WA, :H],
                    ad_fA.rearrange("(w q) h -> q w h", q=128))
                full_b = cfg.nB // 128
                if full_b:
                    ad_fB = tbl_inB[li][:full_b * 128,
                                        L["ad_off"]:L["ad_off"] + H]
                    nc.sync.dma_start(
                        ad_all[:, cfg.WA:cfg.WA + full_b, :H],
                        ad_fB.rearrange("(w q) h -> q w h", q=128))
                if cfg.nB % 128:
                    rem = cfg.nB - full_b * 128
                    nc.sync.dma_start(
                        ad_all[:rem, cfg.WA + full_b, :H],
                        tbl_inB[li][full_b * 128:,
                                    L["ad_off"]:L["ad_off"] + H])

            for j in range(W):
                node_chunk(0, j)
                if j == cfg.WA - 1:
                    emit_ag(0, 0)
            emit_ag(0, 1)
            emit_ad(0)

            # ================= layers =================
            for li in range(3):
                L = Ls[li]
                H, Cc, HC, ROW = L["H"], L["C"], L["HC"], L["row"]
                kch = L["in_ch"] // 128
                NCOL = HC + 2 * H

                # ---- alpha_d for local windows: [128, W, H]
                ad_all = ad_tiles[li]

                if dbg and li == 0:
                    nc.sync.dma_start(dbg_d["d_piece0"][:, :], piece[0][:, :])
                    nc.sync.dma_start(dbg_d["d_ad0"][:, :], ad_all[:, 0, :Ls[0]["H"]])

                # ---- edge phase
                for t in range(W):
                    Tlo, Thi = p.T_lo[t], p.T_hi[t]
                    T = Tlo + Thi + 1
                    nn = cfg.win_size(t)
                    to = p.win_tile_off[t]
                    g = gpool.tile([128, T, ROW], BF16, tag="g")
                    # self-loop rows: direct sequential load, no gather
                    if t < cfg.WA:
                        nc.sync.dma_start(
                            g[:nn, T - 1, :NCOL],
                            tbl_inA[li][t * 128:t * 128 + nn, :NCOL])
                    else:
                        m0 = t * 128 - cfg.nA
                        nc.sync.dma_start(
                            g[:nn, T - 1, :NCOL],
                            tbl_inB[li][m0:m0 + nn, :NCOL])
                    GMAX = 8  # tiles per dma_gather (>1024 idxs crashes HW)
                    for q0 in range(0, Tlo, GMAX):
                        q = min(GMAX, Tlo - q0)
                        nc.gpsimd.dma_gather(
                            g[:, q0:q0 + q, :], tbl_agA[li][:, :],
                            idx_lo[:, p.win_lo_off[t] + q0 * 8:
                                   p.win_lo_off[t] + (q0 + q) * 8],
                            q * 128, q * 128, ROW,
                            queue_num=gq[0] % cfg.n_queues)
                        gq[0] += 1
                    for q0 in range(0, Thi, GMAX):
                        q = min(GMAX, Thi - q0)
                        nc.gpsimd.dma_gather(
                            g[:, Tlo + q0:Tlo + q0 + q, :],
                            tbl_agB[li][:, :],
                            idx_hi[:, p.win_hi_off[t] + q0 * 8:
                                   p.win_hi_off[t] + (q0 + q) * 8],
                            q * 128, q * 128, ROW,
                            queue_num=gq[0] % cfg.n_queues)
                        gq[0] += 1

                    oT = opool.tile([128, T, 128], BF16, tag="oT")
                    oD = opool.tile([128, T, 128], BF16, tag="oD")
                    for k0 in range(0, T, cfg.K_FUSE):
                        K = min(cfg.K_FUSE, T - k0)
                        nc.vector.tensor_tensor(
                            oT[:, k0:k0 + K, :],
                            iota_row.unsqueeze(1).broadcast_to([128, K, 128]),
                            dstcol[:, to + k0:to + k0 + K]
                                .unsqueeze(2).broadcast_to([128, K, 128]),
                            ALU.is_equal)
                        ptd = psC.tile([128, cfg.K_FUSE, 128], BF16, tag="ptd",
                                        bufs=getattr(cfg, "ptd_bufs", 2))
                        for i in range(K):
                            nc.tensor.transpose(ptd[:, i, :],
                                                oT[:, k0 + i, :], ident[:, :])
                        nc.scalar.activation(oD[:, k0:k0 + K, :],
                                             ptd[:, :K, :], AF.Copy)

                    ps_ad = psB.tile([128, MAXT * MAXH], F32, tag="ps_ad")
                    for i in range(T):
                        nc.tensor.matmul(ps_ad[:, i * H:(i + 1) * H],
                                         oD[:, i, :], ad_all[:, t, :H],
                                         start=True, stop=True)
                    e_sb = spool.tile([128, MAXT * MAXH], F32, tag="e_sb")
                    nc.vector.tensor_tensor(
                        e_sb[:, :T * H], ps_ad[:, :T * H],
                        g[:, 0:T, L["as_off"]:L["as_off"] + H],
                        ALU.add)
                    ex1 = spool.tile([128, MAXT * MAXH], F32, tag="ex1")
                    nc.scalar.activation(ex1[:, :T * H], e_sb[:, :T * H], AF.Exp)
                    ex2 = spool.tile([128, MAXT * MAXH], F32, tag="ex2")
                    nc.scalar.activation(ex2[:, :T * H], e_sb[:, :T * H], AF.Exp,
                                         scale=NEG_SLOPE)
                    msg = mpool.tile([128, T, HC + H], BF16, tag="msg")
                    nc.vector.tensor_tensor(
                        msg[:, 0:T, HC:HC + H],
                        ex1[:, :T * H], ex2[:, :T * H], ALU.max)
                    for k0 in range(0, T, cfg.K_FUSE):
                        K = min(cfg.K_FUSE, T - k0)
                        nc.vector.tensor_tensor(
                            msg[:, k0:k0 + K, 0:HC],
                            g[:, k0:k0 + K, 0:HC],
                            msg[:, k0:k0 + K, HC:HC + H]
                                .unsqueeze(3).broadcast_to([128, K, H, Cc]),
                            ALU.mult)
                    ps_w = psA.tile([128, HC + H], F32, tag="ps_w")
                    for i in range(T):
                        nc.tensor.matmul(ps_w[:, :], oT[:, i, :],
                                         msg[:, i, :],
                                         start=(i == 0), stop=(i == T - 1))
                    if dbg and li == 0 and t == 0:
                        nc.sync.dma_start(dbg_d["d_g0"][:, :], g.rearrange("p a b -> p (a b)"))
                        nc.sync.dma_start(dbg_d["d_oT0"][:, :], oT.rearrange("p a b -> p (a b)"))
                        nc.sync.dma_start(dbg_d["d_oD0"][:, :], oD.rearrange("p a b -> p (a b)"))
                        nc.sync.dma_start(dbg_d["d_esb0"][:, :], e_sb[:, :T * H])
                        nc.sync.dma_start(dbg_d["d_msg0"][:, :], msg.rearrange("p a b -> p (a b)"))
                        cw = spool.tile([128, L["HC"] + L["H"]], F32, tag="cw")
                        nc.vector.tensor_copy(cw[:, :], ps_w[:, :])
                        nc.sync.dma_start(dbg_d["d_psw0"][:, :], cw[:, :])
                    rcp = spool.tile([128, MAXH], F32, tag="rcp")
                    nc.vector.reciprocal(rcp[:, :H], ps_w[:, HC:HC + H])
                    if li == 1:
                        nc.vector.tensor_scalar_mul(rcp[:, :H], rcp[:, :H], 0.5)
                    y = spool.tile([128, MAXHC], F32, tag="y")
                    nc.vector.tensor_tensor(
                        y[:, :HC], ps_w[:, :HC],
                        rcp[:, :H].unsqueeze(2).broadcast_to([128, H, Cc]),
                        ALU.mult)
                    if li == 1:
                        nc.vector.tensor_tensor(y[:, :Cc], y[:, :Cc],
                                                y[:, Cc:2 * Cc], ALU.add)
                        ycols = Cc
                    else:
                        ycols = HC
                    nc.vector.tensor_tensor(
                        y[:, :ycols], y[:, :ycols],
                        bg_sb[li][:, :ycols], ALU.add)
                    if li < 2:
                        e1 = spool.tile([128, MAXHC], F32, tag="elu1")
                        nc.scalar.activation(e1[:, :ycols], y[:, :ycols], AF.Exp)
                        nc.scalar.activation(e1[:, :ycols], e1[:, :ycols],
                                             AF.Relu, scale=-1.0, bias=1.0)
                        nc.scalar.activation(y[:, :ycols], y[:, :ycols], AF.Relu)
                        yb = spool.tile([128, MAXHC], BF16, tag="yb")
                        nc.vector.tensor_tensor(yb[:, :ycols], y[:, :ycols],
                                                e1[:, :ycols], ALU.subtract)
                        for k in range(ycols // 128):
                            pt = psC.tile([128, 128], BF16, tag="mm")
                            nc.tensor.transpose(pt[:, :],
                                                yb[:, k * 128:(k + 1) * 128],
                                                ident[:, :])
                            pts = spool.tile([128, 128], BF16, tag="pts")
                            nc.vector.tensor_copy(pts[:, :], pt[:, :])
                            nc.sync.dma_start(
                                piece[li + 1][k * 128:(k + 1) * 128,
                                              t * 128:t * 128 + nn],
                                pts[:, :nn])
                    else:
                        nc.sync.dma_start(out_d[t * 128:t * 128 + nn, :],
                                          y[:nn, :ycols])

                    if li < 2:
                        node_chunk(li + 1, t)
                        if t == cfg.WA - 1:
                            emit_ag(li + 1, 0)
                        if t == W - 1:
                            emit_ag(li + 1, 1)
                            emit_ad(li + 1)
    return nc, out_d


# ---------------------------------------------------------------- runner

def make_inmaps(inputs, cfg: Cfg, p: Plan):
    wts = prep_weights(inputs, cfg)
    x = np.asarray(inputs["x"], np.float32)
    xT = np.ascontiguousarray(x.T).astype(bf)
    n_loc = cfg.n_loc
    in_maps = []
    for c in range(cfg.n_cores):
        m = dict(wts)
        m["xT"] = np.ascontiguousarray(xT[:, c * n_loc:(c + 1) * n_loc])
        m["idx_lo"] = p.idx_lo[c]
        m["idx_hi"] = p.idx_hi[c]
        m["dstcol"] = p.dstloc_col[c]
        in_maps.append(m)
    return in_maps


def build_program(cfg: Cfg, p: Plan, debug=False):
    nc = bacc.Bacc("TRN2", target_bir_lowering=False, debug=debug,
                   num_devices=cfg.n_cores, num_swdge_queues=cfg.n_queues)
    build(nc, cfg, p)
    nc.compile()
    return nc


# ------------------------------------------------------------- entry point

_CACHE = {}


def kernel(**inputs):
    import numpy as _np
    from concourse.bass_utils import run_bass_kernel_spmd

    cfg = Cfg()
    ei = _np.asarray(inputs["edge_index"])
    key = hash(ei.tobytes())
    if key not in _CACHE:
        p = plan_edges(ei, cfg)
        nc = build_program(cfg, p, debug=False)
        _CACHE[key] = (p, nc)
    p, nc = _CACHE[key]
    in_maps = make_inmaps(inputs, cfg, p)
    res = run_bass_kernel_spmd(nc, in_maps, list(range(cfg.n_cores)))
    out = _np.concatenate([res.results[c]["out"] for c in range(cfg.n_cores)], 0)
    return out.astype(_np.float32)



# revision 26
# speedup vs baseline: 1.1040x; 1.1040x over previous
"""Trainium2 Bass kernel: 3-layer GAT message passing, 8-core SPMD.

Dst-sharded edge phase (dma_gather + one-hot scatter matmuls into PSUM;
self-loop rows bulk-loaded, bypassing gather descriptors), node-sharded
table build interleaved into the previous edge phase, split AllGathers.
"""
"""GAT message-passing kernel for TRN2, 8-core SPMD.

Per layer:
  node phase (node-sharded): table rows [h' | alpha_s | alpha_d | pad] for own
  nodes via PE matmuls with host-packed rhs [W | U_s | U_d].
  AllGather table -> full gather table in every core's DRAM.
  edge phase (dst-sharded): per dst-window of 128 nodes, dma_gather rows for
  the window's edges (src-indexed, int16 lo/hi split), build one-hot
  scatter/expand matrices on DVE, expand alpha_d via PE matmul,
  w = max(exp(e), exp(slope*e)) == exp(leaky_relu(e)), weighted messages
  scatter-accumulated into PSUM via one-hot matmul, normalized by the
  softmax denominator (segment max subtraction dropped; shift-invariant).
"""

import math
from dataclasses import dataclass, field

import numpy as np
import ml_dtypes

import concourse.bass as bass
import concourse.bacc as bacc
import concourse.mybir as mybir
import concourse.tile as tile
from concourse import library_config

F32 = mybir.dt.float32
BF16 = mybir.dt.bfloat16
I16 = mybir.dt.int16
F8 = mybir.dt.float8e4
AF = mybir.ActivationFunctionType
ALU = mybir.AluOpType
NEG_SLOPE = 0.2

bf = ml_dtypes.bfloat16


@dataclass
class Cfg:
    N: int = 50000
    n_cores: int = 8
    IN: int = 256
    HID: int = 128
    OUT: int = 128
    TSPLIT: int = 32768
    K_FUSE: int = 8
    n_queues: int = 4

    @property
    def n_loc(self):
        return self.N // self.n_cores

    @property
    def n_win(self):
        return (self.n_loc + 127) // 128

    def win_size(self, t):
        return min(128, self.n_loc - t * 128)

    @property
    def WA(self):
        return (self.n_win + 1) // 2

    @property
    def nA(self):
        return min(self.WA * 128, self.n_loc)

    @property
    def nB(self):
        return self.n_loc - self.nA

    @property
    def a_spl(self):  # window split inside half A (chunk-major AG layout)
        return self.WA * 3 // 4

    @property
    def b_spl(self):  # window split inside half B, relative to half start
        return (self.n_win - self.WA) * 3 // 4

    @property
    def layers(self):
        hid, out = self.HID, self.OUT
        ls = []
        for (H, C, in_ch) in ((4, hid // 2, hid), (2, out, 2 * hid), (1, out, out)):
            HC = H * C
            row = HC + 2 * H
            row_pad = ((row * 2 + 255) // 256) * 256 // 2
            ls.append(dict(H=H, C=C, in_ch=in_ch, HC=HC, row=row_pad,
                           as_off=HC, ad_off=HC + H))
        return ls


# ---------------------------------------------------------------- host plan

@dataclass
class Plan:
    T_lo: list
    T_hi: list
    idx_lo: list
    idx_hi: list
    dstloc_col: list
    oT: list = field(default_factory=list)
    oD: list = field(default_factory=list)
    NT: int = 0
    cols_lo: int = 0
    cols_hi: int = 0
    win_tile_off: list = field(default_factory=list)
    win_lo_off: list = field(default_factory=list)
    win_hi_off: list = field(default_factory=list)


def plan_edges(edge_index, cfg: Cfg) -> Plan:
    N, C = cfg.N, cfg.n_cores
    n_loc = cfg.n_loc
    src = np.asarray(edge_index[0], np.int64)      # self loops handled by a
    dst = np.asarray(edge_index[1], np.int64)      # direct per-window load
    core = dst // n_loc
    win = (dst % n_loc) // 128
    dloc = (dst % n_loc) % 128
    s_core = src // n_loc
    s_loc = src % n_loc
    lo = s_loc < cfg.nA
    # chunk-major AG table layout: [C*a1 rows of chunk1 | C*(nA-a1) of chunk2]
    a1 = cfg.a_spl * 128
    idx_a = np.where(s_loc < a1, s_core * a1 + s_loc,
                     C * a1 + s_core * (cfg.nA - a1) + (s_loc - a1))
    bloc = s_loc - cfg.nA
    b1 = cfg.b_spl * 128
    idx_b = np.where(bloc < b1, s_core * b1 + bloc,
                     C * b1 + s_core * (cfg.nB - b1) + (bloc - b1))

    W = cfg.n_win
    buckets = {}
    for c in range(C):
        m_c = core == c
        for t in range(W):
            m = m_c & (win == t)
            ml = m & lo
            mh = m & ~lo
            sl, dll = idx_a[ml], dloc[ml]
            o = np.argsort(sl, kind="stable")
            buckets[(c, t, 0)] = (sl[o], dll[o])
            sh, dlh = idx_b[mh], dloc[mh]
            o = np.argsort(sh, kind="stable")
            buckets[(c, t, 1)] = (sh[o], dlh[o])

    T_lo = [max(math.ceil(len(buckets[(c, t, 0)][0]) / 128) for c in range(C))
            for t in range(W)]
    T_hi = [max(math.ceil(len(buckets[(c, t, 1)][0]) / 128) for c in range(C))
            for t in range(W)]

    NT = sum(T_lo) + sum(T_hi) + W  # +1 self tile per window
    p = Plan(T_lo=T_lo, T_hi=T_hi, idx_lo=[], idx_hi=[], dstloc_col=[],
             NT=NT, cols_lo=max(sum(T_lo) * 8, 8),
             cols_hi=max(sum(T_hi) * 8, 8))
    off = olo = ohi = 0
    for t in range(W):
        p.win_tile_off.append(off)
        p.win_lo_off.append(olo)
        p.win_hi_off.append(ohi)
        off += T_lo[t] + T_hi[t] + 1
        olo += T_lo[t] * 8
        ohi += T_hi[t] * 8

    for c in range(C):
        ilo = np.zeros((128, p.cols_lo), np.int16)
        ihi = np.zeros((128, p.cols_hi), np.int16)
        dcol = np.full((128, NT), -1.0, np.float32)
        for t in range(W):
            for half, (idx_arr, col_off, Tn) in enumerate(
                ((ilo, p.win_lo_off[t], T_lo[t]),
                 (ihi, p.win_hi_off[t], T_hi[t]))):
                if Tn == 0:
                    continue
                s, dl = buckets[(c, t, half)]
                n = Tn * 128
                si = np.zeros(n, np.int64)
                si[:len(s)] = s
                dli = np.full(n, -1.0, np.float32)
                dli[:len(dl)] = dl
                blk = si.astype(np.int16).reshape(Tn * 8, 16).T
                idx_arr[:, col_off:col_off + Tn * 8] = np.tile(blk, (8, 1))
                tb = p.win_tile_off[t] + (0 if half == 0 else T_lo[t])
                dcol[:, tb:tb + Tn] = dli.reshape(Tn, 128).T
            ts = p.win_tile_off[t] + T_lo[t] + T_hi[t]
            nn_w = min(128, n_loc - t * 128)
            selfcol = np.full(128, -1.0, np.float32)
            selfcol[:nn_w] = np.arange(nn_w)
            dcol[:, ts] = selfcol
        p.idx_lo.append(ilo)
        p.idx_hi.append(ihi)
        p.dstloc_col.append(dcol.astype(bf))
        # host-built one-hot scatter (oT) / expand (oD) matrices, fp8
        f8 = np.dtype(ml_dtypes.float8_e4m3)
        ar = np.arange(128, dtype=np.float32)
        eq = (dcol[:, :, None] == ar[None, None, :])  # [p, tile, d]
        p.oT.append(np.ascontiguousarray(
            eq.reshape(128, NT * 128).astype(np.float32)).astype(f8))
        p.oD.append(np.ascontiguousarray(
            eq.transpose(2, 1, 0).reshape(128, NT * 128)
            .astype(np.float32)).astype(f8))
    return p


# ------------------------------------------------------------- host weights

def prep_weights(inp, cfg: Cfg):
    out = {}
    for li, (wk, ak, dk) in enumerate(
            (("g1_W", "g1_as", "g1_ad"), ("g2_W", "g2_as", "g2_ad"),
             ("g3_W", "g3_as", "g3_ad"))):
        L = cfg.layers[li]
        Wm = np.asarray(inp[wk], np.float32)
        a_s = np.asarray(inp[ak], np.float32)
        a_d = np.asarray(inp[dk], np.float32)
        H, Cc = L["H"], L["C"]
        U_s = np.stack([Wm[:, h * Cc:(h + 1) * Cc] @ a_s[h] for h in range(H)], 1)
        U_d = np.stack([Wm[:, h * Cc:(h + 1) * Cc] @ a_d[h] for h in range(H)], 1)
        out[f"WG{li+1}"] = np.concatenate([Wm, U_s, U_d], 1).astype(bf)
        out[f"bG{li+1}"] = np.ascontiguousarray(np.broadcast_to(
            np.asarray(inp[f"g{li+1}_b"], np.float32)[None, :],
            (128, len(inp[f"g{li+1}_b"]))))
    out["Wm1"] = np.asarray(inp["W1"], np.float32).astype(bf)
    out["Wm2"] = np.asarray(inp["W2"], np.float32).astype(bf)
    out["b1c"] = np.ascontiguousarray(np.asarray(inp["b1"], np.float32)[:, None])
    out["b2c"] = np.ascontiguousarray(np.asarray(inp["b2"], np.float32)[:, None])
    out["ident"] = np.eye(128, dtype=np.float32).astype(bf)
    return out


# ---------------------------------------------------------------- builder

def build(nc, cfg: Cfg, p: Plan, dbg=False):
    W = cfg.n_win
    n_loc = cfg.n_loc
    Ls = cfg.layers
    MAXT = max(p.T_lo[t] + p.T_hi[t] for t in range(W)) + 1
    MAXHC = max(L["HC"] for L in Ls)
    MAXH = max(L["H"] for L in Ls)

    def din(name, shape, dt):
        return nc.dram_tensor(name, list(shape), dt, kind="ExternalInput")

    xT = din("xT", (cfg.IN, n_loc), BF16)
    Wm1 = din("Wm1", (cfg.IN, cfg.HID), BF16)
    Wm2 = din("Wm2", (cfg.HID, cfg.HID), BF16)
    b1c = din("b1c", (cfg.HID, 1), F32)
    b2c = din("b2c", (cfg.HID, 1), F32)
    WG = [din(f"WG{i+1}", (Ls[i]["in_ch"], Ls[i]["HC"] + 2 * Ls[i]["H"]), BF16)
          for i in range(3)]
    bG = [din(f"bG{i+1}", (128, Ls[i]["HC"] if i == 0 else Ls[i]["C"]), F32)
          for i in range(3)]
    ident_d = din("ident", (128, 128), BF16)
    idx_lo_d = din("idx_lo", (128, p.cols_lo), I16)
    idx_hi_d = din("idx_hi", (128, p.cols_hi), I16)
    oT_d = din("oT", (128, p.NT * 128), F8)
    oD_d = din("oD", (128, p.NT * 128), F8)
    out_d = nc.dram_tensor("out", [n_loc, cfg.OUT], F32, kind="ExternalOutput")
    dbg_d = {}
    if dbg:
        L0 = Ls[0]
        T0 = p.T_lo[0] + p.T_hi[0]
        for nm, shape in (("d_piece0", (cfg.HID, n_loc)),
                          ("d_tin0", (n_loc, L0["row"])),
                          ("d_tag0", (cfg.N, L0["row"])),
                          ("d_g0", (128, T0 * L0["row"])),
                          ("d_oT0", (128, T0 * 128)),
                          ("d_oD0", (128, T0 * 128)),
                          ("d_esb0", (128, T0 * L0["H"])),
                          ("d_msg0", (128, T0 * (L0["HC"] + L0["H"]))),
                          ("d_psw0", (128, L0["HC"] + L0["H"])),
                          ("d_ad0", (128, L0["H"]))):
            dt = BF16 if nm in ("d_tin0", "d_tag0", "d_g0", "d_oT0", "d_oD0",
                                "d_msg0", "d_piece0", "d_ad0") else F32
            dbg_d[nm] = nc.dram_tensor(nm, list(shape), dt, kind="ExternalOutput")

    rep = [list(range(cfg.n_cores))]

    with tile.TileContext(nc) as tc:
        with (
            tc.tile_pool(name="const", bufs=1) as cpool,
            tc.tile_pool(name="gat", bufs=3) as gpool,
            tc.tile_pool(name="oh", bufs=3) as opool,
            tc.tile_pool(name="msg", bufs=2) as mpool,
            tc.tile_pool(name="small", bufs=4) as spool,
            tc.tile_pool(name="nodes", bufs=3) as npool,
            tc.tile_pool(name="psA", bufs=getattr(cfg, "psA_bufs", 2), space="PSUM") as psA,
            tc.tile_pool(name="psB", bufs=getattr(cfg, "psB_bufs", 2), space="PSUM") as psB,
            tc.tile_pool(name="psC", bufs=2, space="PSUM") as psC,
            tc.tile_pool(name="dram", bufs=1, space="DRAM") as dpool,
        ):
            nc.gpsimd.load_library(library_config.mlp)
            gq = [0]  # round-robin SWDGE queue counter for gathers

            def load_const(handle, shape, dtp, tag):
                t = cpool.tile(list(shape), dtp, tag=tag, name=tag)
                nc.sync.dma_start(t[:], handle[:])
                return t

            ident = load_const(ident_d, (128, 128), BF16, "ident")
            idx_lo = load_const(idx_lo_d, (128, p.cols_lo), I16, "idx_lo")
            idx_hi = load_const(idx_hi_d, (128, p.cols_hi), I16, "idx_hi")
            wg_sb = []
            for i in range(3):
                L = Ls[i]
                kch = L["in_ch"] // 128
                t = cpool.tile([128, kch, L["HC"] + 2 * L["H"]], BF16, tag=f"wg{i}",
                               name=f"wg{i}")
                for k in range(kch):
                    nc.sync.dma_start(t[:, k, :], WG[i][k * 128:(k + 1) * 128, :])
                wg_sb.append(t)
            bg_sb = [load_const(bG[i], (128, Ls[i]["HC"] if i == 0 else Ls[i]["C"]),
                                F32, f"bg{i}") for i in range(3)]
            wm1 = cpool.tile([128, 2, cfg.HID], BF16, tag="wm1")
            for k in range(2):
                nc.sync.dma_start(wm1[:, k, :], Wm1[k * 128:(k + 1) * 128, :])
            wm2 = load_const(Wm2, (cfg.HID, cfg.HID), BF16, "wm2")
            b1s = load_const(b1c, (cfg.HID, 1), F32, "b1s")
            b2s = load_const(b2c, (cfg.HID, 1), F32, "b2s")

            piece = [dpool.tile([cfg.HID, n_loc], BF16, tag="piece0",
                                name="piece0"),
                     dpool.tile([Ls[0]["HC"], n_loc], BF16, tag="piece1",
                                name="piece1"),
                     dpool.tile([Ls[1]["C"], n_loc], BF16, tag="piece2",
                                name="piece2")]
            tbl_inA = [dpool.tile([cfg.nA, Ls[i]["row"]], BF16, tag=f"tinA{i}",
                                  name=f"tinA{i}")
                       for i in range(3)]
            tbl_inB = [dpool.tile([cfg.nB, Ls[i]["row"]], BF16, tag=f"tinB{i}",
                                  name=f"tinB{i}")
                       for i in range(3)]
            tbl_agA = [dpool.tile([cfg.nA * cfg.n_cores, Ls[i]["row"]], BF16,
                                  tag=f"tagA{i}", name=f"tagA{i}")
                       for i in range(3)]
            tbl_agB = [dpool.tile([cfg.nB * cfg.n_cores, Ls[i]["row"]], BF16,
                                  tag=f"tagB{i}", name=f"tagB{i}")
                       for i in range(3)]

            # ================= MLP (node-sharded) =================
            CH = 512
            nch = math.ceil(n_loc / CH)
            for j in range(nch):
                n0 = j * CH
                nn = min(CH, n_loc - n0)
                xt = npool.tile([128, 2, CH], BF16, tag="xt")
                for k in range(2):
                    nc.sync.dma_start(xt[:, k, :nn],
                                      xT[k * 128:(k + 1) * 128, n0:n0 + nn])
                ps = psC.tile([128, CH], F32, tag="mm")
                for k in range(2):
                    nc.tensor.matmul(ps[:, :nn], wm1[:, k, :], xt[:, k, :nn],
                                     start=(k == 0), stop=(k == 1))
                h1 = npool.tile([128, CH], BF16, tag="h1")
                nc.scalar.activation(h1[:, :nn], ps[:, :nn], AF.Relu,
                                     bias=b1s[:, 0:1])
                ps2 = psC.tile([128, CH], F32, tag="mm")
                nc.tensor.matmul(ps2[:, :nn], wm2[:, :], h1[:, :nn],
                                 start=True, stop=True)
                h2 = npool.tile([128, CH], BF16, tag="h2")
                nc.scalar.activation(h2[:, :nn], ps2[:, :nn], AF.Relu,
                                     bias=b2s[:, 0:1])
                nc.sync.dma_start(piece[0][:, n0:n0 + nn], h2[:, :nn])

            ad_tiles = {}

            def node_chunk(li, j):
                L = Ls[li]
                kch = L["in_ch"] // 128
                NCOL = L["HC"] + 2 * L["H"]
                ROW = L["row"]
                n0 = j * 128
                nn = cfg.win_size(j)
                lh = npool.tile([128, kch, 128], BF16, tag="lh", name="lh")
                for k in range(kch):
                    nc.sync.dma_start(
                        lh[:, k, :nn],
                        piece[li][k * 128:(k + 1) * 128, n0:n0 + nn])
                ps = psC.tile([128, NCOL], F32, tag="mm", name="psn")
                for k in range(kch):
                    nc.tensor.matmul(ps[:nn, :], lh[:, k, :nn],
                                     wg_sb[li][:, k, :],
                                     start=(k == 0), stop=(k == kch - 1))
                tb = npool.tile([128, ROW], BF16, tag="tb", name="tb")
                nc.scalar.activation(tb[:nn, :NCOL], ps[:nn, :], AF.Copy)
                if j < cfg.WA:
                    nc.sync.dma_start(
                        tbl_inA[li][n0:n0 + nn, :NCOL], tb[:nn, :NCOL])
                else:
                    m0 = n0 - cfg.nA
                    nc.sync.dma_start(
                        tbl_inB[li][m0:m0 + nn, :NCOL], tb[:nn, :NCOL])

            def emit_ag(li, half, w0, w1):
                # AllGather rows [w0*128, w1*128) of the half's local table.
                # Chunk-major gathered layout: this chunk's 8 rank-shards land
                # contiguously at [C*n0, C*n1) (idx arrays match; plan_edges).
                tin = tbl_inA[li] if half == 0 else tbl_inB[li]
                tag = tbl_agA[li] if half == 0 else tbl_agB[li]
                nh = cfg.nA if half == 0 else cfg.nB
                n0 = w0 * 128
                n1 = min(w1 * 128, nh)
                if n0 >= n1:
                    return
                nc.gpsimd.collective_compute(
                    "AllGather", ALU.bypass, replica_groups=rep,
                    ins=[tin[n0:n1, :]],
                    outs=[tag[cfg.n_cores * n0:cfg.n_cores * n1, :]])

            def emit_ad(li):
                L = Ls[li]
                H = L["H"]
                ad_all = spool.tile([128, W, MAXH], BF16, tag="ad_all",
                                    name="ad_all", bufs=2)
                ad_tiles[li] = ad_all
                nc.vector.memset(ad_all[:], 0.0)
                ad_fA = tbl_inA[li][:, L["ad_off"]:L["ad_off"] + H]
                nc.sync.dma_start(
                    ad_all[:, :cfg.WA, :H],
                    ad_fA.rearrange("(w q) h -> q w h", q=128))
                full_b = cfg.nB // 128
                if full_b:
                    ad_fB = tbl_inB[li][:full_b * 128,
                                        L["ad_off"]:L["ad_off"] + H]
                    nc.sync.dma_start(
                        ad_all[:, cfg.WA:cfg.WA + full_b, :H],
                        ad_fB.rearrange("(w q) h -> q w h", q=128))
                if cfg.nB % 128:
                    rem = cfg.nB - full_b * 128
                    nc.sync.dma_start(
                        ad_all[:rem, cfg.WA + full_b, :H],
                        tbl_inB[li][full_b * 128:,
                                    L["ad_off"]:L["ad_off"] + H])

            A_SPL = cfg.a_spl
            B_SPL = cfg.b_spl  # relative to half-B start

            for j in range(W):
                node_chunk(0, j)
                if j == A_SPL - 1:
                    emit_ag(0, 0, 0, A_SPL)
                elif j == cfg.WA - 1:
                    emit_ag(0, 0, A_SPL, cfg.WA)
                elif j == cfg.WA + B_SPL - 1:
                    emit_ag(0, 1, 0, B_SPL)
            emit_ag(0, 1, B_SPL, W - cfg.WA)
            emit_ad(0)

            # ================= layers =================
            for li in range(3):
                L = Ls[li]
                H, Cc, HC, ROW = L["H"], L["C"], L["HC"], L["row"]
                kch = L["in_ch"] // 128
                NCOL = HC + 2 * H

                # ---- alpha_d for local windows: [128, W, H]
                ad_all = ad_tiles[li]

                if dbg and li == 0:
                    nc.sync.dma_start(dbg_d["d_piece0"][:, :], piece[0][:, :])
                    nc.sync.dma_start(dbg_d["d_ad0"][:, :], ad_all[:, 0, :Ls[0]["H"]])

                # ---- edge phase
                for t in range(W):
                    Tlo, Thi = p.T_lo[t], p.T_hi[t]
                    T = Tlo + Thi + 1
                    nn = cfg.win_size(t)
                    to = p.win_tile_off[t]
                    g = gpool.tile([128, T, ROW], BF16, tag="g")
                    # self-loop rows: direct sequential load, no gather
                    if t < cfg.WA:
                        nc.sync.dma_start(
                            g[:nn, T - 1, :NCOL],
                            tbl_inA[li][t * 128:t * 128 + nn, :NCOL])
                    else:
                        m0 = t * 128 - cfg.nA
                        nc.sync.dma_start(
                            g[:nn, T - 1, :NCOL],
                            tbl_inB[li][m0:m0 + nn, :NCOL])
                    GMAX = 8  # tiles per dma_gather (>1024 idxs crashes HW)
                    for q0 in range(0, Tlo, GMAX):
                        q = min(GMAX, Tlo - q0)
                        nc.gpsimd.dma_gather(
                            g[:, q0:q0 + q, :], tbl_agA[li][:, :],
                            idx_lo[:, p.win_lo_off[t] + q0 * 8:
                                   p.win_lo_off[t] + (q0 + q) * 8],
                            q * 128, q * 128, ROW,
                            queue_num=gq[0] % cfg.n_queues)
                        gq[0] += 1
                    for q0 in range(0, Thi, GMAX):
                        q = min(GMAX, Thi - q0)
                        nc.gpsimd.dma_gather(
                            g[:, Tlo + q0:Tlo + q0 + q, :],
                            tbl_agB[li][:, :],
                            idx_hi[:, p.win_hi_off[t] + q0 * 8:
                                   p.win_hi_off[t] + (q0 + q) * 8],
                            q * 128, q * 128, ROW,
                            queue_num=gq[0] % cfg.n_queues)
                        gq[0] += 1

                    oT = opool.tile([128, T, 128], F8, tag="oT")
                    oD = opool.tile([128, T, 128], F8, tag="oD")
                    nc.sync.dma_start(
                        oT.rearrange("p a b -> p (a b)"),
                        oT_d[:, to * 128:(to + T) * 128])
                    nc.sync.dma_start(
                        oD.rearrange("p a b -> p (a b)"),
                        oD_d[:, to * 128:(to + T) * 128])

                    ps_ad = psB.tile([128, MAXT * MAXH], F32, tag="ps_ad")
                    for i in range(T):
                        nc.tensor.matmul(ps_ad[:, i * H:(i + 1) * H],
                                         oD[:, i, :], ad_all[:, t, :H],
                                         start=True, stop=True)
                    e_sb = spool.tile([128, MAXT * MAXH], F32, tag="e_sb")
                    nc.vector.tensor_tensor(
                        e_sb[:, :T * H], ps_ad[:, :T * H],
                        g[:, 0:T, L["as_off"]:L["as_off"] + H],
                        ALU.add)
                    ex1 = spool.tile([128, MAXT * MAXH], F32, tag="ex1")
                    nc.scalar.activation(ex1[:, :T * H], e_sb[:, :T * H], AF.Exp)
                    ex2 = spool.tile([128, MAXT * MAXH], F32, tag="ex2")
                    nc.scalar.activation(ex2[:, :T * H], e_sb[:, :T * H], AF.Exp,
                                         scale=NEG_SLOPE)
                    msg = mpool.tile([128, T, HC + H], BF16, tag="msg")
                    nc.vector.tensor_tensor(
                        msg[:, 0:T, HC:HC + H],
                        ex1[:, :T * H], ex2[:, :T * H], ALU.max)
                    for k0 in range(0, T, cfg.K_FUSE):
                        K = min(cfg.K_FUSE, T - k0)
                        nc.vector.tensor_tensor(
                            msg[:, k0:k0 + K, 0:HC],
                            g[:, k0:k0 + K, 0:HC],
                            msg[:, k0:k0 + K, HC:HC + H]
                                .unsqueeze(3).broadcast_to([128, K, H, Cc]),
                            ALU.mult)
                    ps_w = psA.tile([128, HC + H], F32, tag="ps_w")
                    for i in range(T):
                        nc.tensor.matmul(ps_w[:, :], oT[:, i, :],
                                         msg[:, i, :],
                                         start=(i == 0), stop=(i == T - 1))
                    if dbg and li == 0 and t == 0:
                        nc.sync.dma_start(dbg_d["d_g0"][:, :], g.rearrange("p a b -> p (a b)"))
                        nc.sync.dma_start(dbg_d["d_oT0"][:, :], oT.rearrange("p a b -> p (a b)"))
                        nc.sync.dma_start(dbg_d["d_oD0"][:, :], oD.rearrange("p a b -> p (a b)"))
                        nc.sync.dma_start(dbg_d["d_esb0"][:, :], e_sb[:, :T * H])
                        nc.sync.dma_start(dbg_d["d_msg0"][:, :], msg.rearrange("p a b -> p (a b)"))
                        cw = spool.tile([128, L["HC"] + L["H"]], F32, tag="cw")
                        nc.vector.tensor_copy(cw[:, :], ps_w[:, :])
                        nc.sync.dma_start(dbg_d["d_psw0"][:, :], cw[:, :])
                    rcp = spool.tile([128, MAXH], F32, tag="rcp")
                    nc.vector.reciprocal(rcp[:, :H], ps_w[:, HC:HC + H])
                    if li == 1:
                        nc.scalar.activation(rcp[:, :H], rcp[:, :H], AF.Copy,
                                             scale=0.5)
                    y = spool.tile([128, MAXHC], F32, tag="y")
                    nc.vector.tensor_tensor(
                        y[:, :HC], ps_w[:, :HC],
                        rcp[:, :H].unsqueeze(2).broadcast_to([128, H, Cc]),
                        ALU.mult)
                    if li == 1:
                        nc.vector.tensor_tensor(y[:, :Cc], y[:, :Cc],
                                                y[:, Cc:2 * Cc], ALU.add)
                        ycols = Cc
                    else:
                        ycols = HC
                    nc.vector.tensor_tensor(
                        y[:, :ycols], y[:, :ycols],
                        bg_sb[li][:, :ycols], ALU.add)
                    if li < 2:
                        e1 = spool.tile([128, MAXHC], F32, tag="elu1")
                        nc.scalar.activation(e1[:, :ycols], y[:, :ycols], AF.Exp)
                        nc.scalar.activation(e1[:, :ycols], e1[:, :ycols],
                                             AF.Relu, scale=-1.0, bias=1.0)
                        nc.scalar.activation(y[:, :ycols], y[:, :ycols], AF.Relu)
                        yb = spool.tile([128, MAXHC], BF16, tag="yb")
                        nc.vector.tensor_tensor(yb[:, :ycols], y[:, :ycols],
                                                e1[:, :ycols], ALU.subtract)
                        for k in range(ycols // 128):
                            pt = psC.tile([128, 128], BF16, tag="mm")
                            nc.tensor.transpose(pt[:, :],
                                                yb[:, k * 128:(k + 1) * 128],
                                                ident[:, :])
                            pts = spool.tile([128, 128], BF16, tag="pts")
                            nc.scalar.activation(pts[:, :], pt[:, :], AF.Copy)
                            nc.sync.dma_start(
                                piece[li + 1][k * 128:(k + 1) * 128,
                                              t * 128:t * 128 + nn],
                                pts[:, :nn])
                    else:
                        nc.sync.dma_start(out_d[t * 128:t * 128 + nn, :],
                                          y[:nn, :ycols])

                    if li < 2:
                        node_chunk(li + 1, t)
                        if t == A_SPL - 1:
                            emit_ag(li + 1, 0, 0, A_SPL)
                        elif t == cfg.WA - 1:
                            emit_ag(li + 1, 0, A_SPL, cfg.WA)
                        elif t == cfg.WA + B_SPL - 1:
                            emit_ag(li + 1, 1, 0, B_SPL)
                        if t == W - 1:
                            emit_ag(li + 1, 1, B_SPL, W - cfg.WA)
                            emit_ad(li + 1)
    return nc, out_d


# ---------------------------------------------------------------- runner

def make_inmaps(inputs, cfg: Cfg, p: Plan):
    wts = prep_weights(inputs, cfg)
    x = np.asarray(inputs["x"], np.float32)
    xT = np.ascontiguousarray(x.T).astype(bf)
    n_loc = cfg.n_loc
    in_maps = []
    for c in range(cfg.n_cores):
        m = dict(wts)
        m["xT"] = np.ascontiguousarray(xT[:, c * n_loc:(c + 1) * n_loc])
        m["idx_lo"] = p.idx_lo[c]
        m["idx_hi"] = p.idx_hi[c]
        m["oT"] = p.oT[c]
        m["oD"] = p.oD[c]
        in_maps.append(m)
    return in_maps


def build_program(cfg: Cfg, p: Plan, debug=False):
    nc = bacc.Bacc("TRN2", target_bir_lowering=False, debug=debug,
                   num_devices=cfg.n_cores, num_swdge_queues=cfg.n_queues)
    build(nc, cfg, p)
    nc.compile()
    return nc


# ------------------------------------------------------------- entry point

_CACHE = {}


def kernel(**inputs):
    import numpy as _np
    from concourse.bass_utils import run_bass_kernel_spmd

    cfg = Cfg()
    ei = _np.asarray(inputs["edge_index"])
    key = hash(ei.tobytes())
    if key not in _CACHE:
        p = plan_edges(ei, cfg)
        nc = build_program(cfg, p, debug=False)
        _CACHE[key] = (p, nc)
    p, nc = _CACHE[key]
    in_maps = make_inmaps(inputs, cfg, p)
    res = run_bass_kernel_spmd(nc, in_maps, list(range(cfg.n_cores)))
    out = _np.concatenate([res.results[c]["out"] for c in range(cfg.n_cores)], 0)
    return out.astype(_np.float32)

